# revision 11
# baseline (speedup 1.0000x reference)
"""GQA cross-attention block on 8 trn2 NeuronCores.

Sharding: tensor-parallel over heads. Core c owns KV group g=c (64 dims of
K/V) and its 4 query heads (256 q channels). Each core computes its heads'
attention plus its slice of the o-projection (rows c*256:(c+1)*256 of Wo),
producing a full-shape partial output; the host sums the 8 partials and
adds bo.

Structure (v3):
  - K/V projection packed: stationary [128h, 128] = [Wk_h | Wv_h] ->
    kvT [128, S] (K rows 0:64, V rows 64:128). Evacuated on DVE
    (tensor_scalar_add) so ACT stays reserved for exp.
  - Scores row-tiled 2x on the PE: kTd [128, S] holds K duplicated on both
    partition halves; qd2 [128, S] holds a HEAD PAIR. Two concurrent K=64
    matmuls (tile_position (0,0)/(64,0)) fill one [128, 1024] PSUM pair ->
    a single [128, 1024] exp on ACT (the kernel bottleneck: ~220us/core of
    pure exp streaming).
  - AV via v_aug [128, 65] (ones column -> softmax denominator Z free in
    row 64). Z rows batched into one [128, 32] reciprocal via DMA
    gather/scatter; 1/Z broadcast with a K=1 PE matmul; DVE mul -> oT.
  - Cross-batch software pipelining by interleaved EMISSION: batch 1's
    projection work is emitted in small chunks inside batch 0's attention
    kc-loop (and batch 0's o-projection inside batch 1's attention), so
    the Tile scheduler's priority order alternates and the PE fills the
    ACT-bound gaps. PSUM tags: sc 4 banks, av 2, prj 1, pr 1.
  - DMA spread: enc on gpsimd, x on sync(+scalar at startup), weights on
    scalar, z-dance + stores on sync.
"""

import numpy as np
import ml_dtypes

import concourse.bass as bass
from concourse import bacc
import concourse.mybir as mybir
import concourse.tile as tile
from concourse.bass_utils import run_bass_kernel_spmd
from concourse.masks import make_identity

BF16 = ml_dtypes.bfloat16
F32 = mybir.dt.float32
BF = mybir.dt.bfloat16

B = 2
S = 2048
HID = 2048
D = 64          # head dim
CH = 4 * D      # 256 q channels per core
NCORES = 8
NH = HID // 128  # 16 hidden chunks
NKC = S // 128   # 16 key chunks of 128
NQC = S // 512   # 4 q tiles of 512
NST = S // 512   # 4 s tiles of 512
SCALE = 1.0 / np.sqrt(D)


def _build_nc() -> bass.Bass:
    nc = bacc.Bacc()

    xT = nc.dram_tensor("xT", [B, HID, S], BF, kind="ExternalInput")
    encT = nc.dram_tensor("encT", [B, HID, S], BF, kind="ExternalInput")
    wq = nc.dram_tensor("wq", [HID, CH], BF, kind="ExternalInput")
    wkv = nc.dram_tensor("wkv", [HID, 128], BF, kind="ExternalInput")
    wo = nc.dram_tensor("wo", [CH, HID], BF, kind="ExternalInput")
    bq = nc.dram_tensor("bq", [CH, 1], F32, kind="ExternalInput")
    bkv = nc.dram_tensor("bkv", [128, 1], F32, kind="ExternalInput")
    out = nc.dram_tensor("out", [B, S, HID], BF, kind="ExternalOutput")

    EXP = mybir.ActivationFunctionType.Exp

    with tile.TileContext(nc) as tc:
        with (
            tc.tile_pool(name="wpool", bufs=1) as wpool,
            tc.tile_pool(name="io", bufs=22) as io_pool,
            tc.tile_pool(name="acts", bufs=2) as acts,
            tc.tile_pool(name="vaug", bufs=2 * NKC) as vaug_pool,
            tc.tile_pool(name="epool", bufs=3) as epool,
            tc.tile_pool(name="avsb", bufs=8) as avsb_pool,
            tc.tile_pool(name="zp", bufs=2) as zpool,
            tc.tile_pool(name="osb", bufs=2) as osb_pool,
            tc.tile_pool(name="ps_sc", bufs=2, space="PSUM") as ps_sc,
            tc.tile_pool(name="ps_av", bufs=2, space="PSUM") as ps_av,
            tc.tile_pool(name="ps_prj", bufs=1, space="PSUM") as ps_prj,
            tc.tile_pool(name="ps_pr", bufs=1, space="PSUM") as ps_pr,
        ):
            # ---- resident weights / constants (scalar HWDGE queue) ----
            wkv_t = []
            for h in range(NH):
                wkvh = wpool.tile([128, 128], BF, name=f"wkv{h}")
                nc.scalar.dma_start(out=wkvh[:], in_=wkv[h * 128:(h + 1) * 128, :])
                wkv_t.append(wkvh)
            bkv_t = wpool.tile([128, 1], F32, name="bkv_t")
            nc.scalar.dma_start(out=bkv_t[:], in_=bkv[:, :])
            ident = wpool.tile([128, 128], BF, name="ident")
            make_identity(nc, ident[:])
            wq_t = []
            for h in range(NH):
                wqh = wpool.tile([128, CH], BF, name=f"wq{h}")
                nc.scalar.dma_start(out=wqh[:], in_=wq[h * 128:(h + 1) * 128, :])
                wq_t.append(wqh)
            bq_t = []
            for cc in range(2):
                bqc = wpool.tile([128, 1], F32, name=f"bq{cc}")
                nc.scalar.dma_start(out=bqc[:], in_=bq[cc * 128:(cc + 1) * 128, :])
                bq_t.append(bqc)
            wo_t = []
            for cc in range(2):
                woc = wpool.tile([128, HID], BF, name=f"wo{cc}")
                nc.scalar.dma_start(out=woc[:], in_=wo[cc * 128:(cc + 1) * 128, :])
                wo_t.append(woc)
            ones1 = wpool.tile([1, D], BF, name="ones1")
            nc.gpsimd.memset(ones1[:], 1.0)

            state = {}

            def proj_phase(b, startup):
                """Generator: KV proj, kTd/vT dup, v_aug, Q proj for batch b.
                Yields between small chunks so it can be pumped as PE filler
                inside the other batch's attention loop."""
                st_ = {}
                state[b] = st_
                # --- KV projection ---
                kvT = acts.tile([128, S], BF, tag="kvT", bufs=1, name=f"kvT{b}")
                for st in range(NST):
                    ssl = slice(st * 512, (st + 1) * 512)
                    ets = []
                    for h in range(NH):
                        et = io_pool.tile([128, 512], BF, tag="io",
                                          name=f"es{b}{st}{h}")
                        eng = nc.gpsimd if h % 2 == 0 else (nc.sync if startup else nc.gpsimd)
                        eng.dma_start(
                            out=et[:], in_=encT[b, h * 128:(h + 1) * 128, ssl])
                        ets.append(et)
                    yield
                    kvps = ps_prj.tile([128, 512], F32, tag="prj",
                                       name=f"kvp{b}{st}")
                    for h in range(NH):
                        nc.tensor.matmul(
                            kvps[:], wkv_t[h][:], ets[h][:],
                            start=(h == 0), stop=(h == NH - 1))
                        if h % 4 == 3:
                            yield
                    nc.vector.tensor_scalar_add(kvT[:, ssl], kvps[:], bkv_t[:])
                    yield
                # --- kTd (K duplicated on both halves), vT ---
                kTd = acts.tile([128, S], BF, tag="kTd", name=f"kTd{b}")
                vT = acts.tile([D, S], BF, tag="vT", bufs=1, name=f"vT{b}")
                nc.gpsimd.dma_start(out=kTd[0:D, :], in_=kvT[0:D, :])
                nc.gpsimd.dma_start(out=kTd[D:128, :], in_=kvT[0:D, :])
                nc.gpsimd.dma_start(out=vT[:], in_=kvT[D:128, :])
                st_["kTd"] = kTd
                yield
                # --- v_aug chunks [128, 65] with ones in col 64 ---
                v_aug = []
                for kc in range(NKC):
                    vtp = ps_pr.tile([128, D], BF, tag="pr", name=f"vtp{b}{kc}")
                    nc.tensor.transpose(
                        vtp[:], vT[:, kc * 128:(kc + 1) * 128], ident[0:D, 0:D])
                    va = vaug_pool.tile([128, D + 1], BF, tag=f"va{kc}",
                                        name=f"va{b}{kc}")
                    nc.gpsimd.memset(va[:, D:D + 1], 1.0)
                    nc.vector.tensor_copy(va[:, 0:D], vtp[:])
                    v_aug.append(va)
                    if kc % 4 == 3:
                        yield
                st_["va"] = v_aug
                # --- Q projection -> head-pair tiles qd2[hp] ---
                qd2 = [
                    acts.tile([128, S], BF, tag=f"qd{hp}", name=f"qd{b}{hp}")
                    for hp in range(2)
                ]
                st_["qd2"] = qd2
                for st in range(NST):
                    ssl = slice(st * 512, (st + 1) * 512)
                    xts = []
                    for h in range(NH):
                        xt = io_pool.tile([128, 512], BF, tag="io",
                                          name=f"xs{b}{st}{h}")
                        eng = nc.sync if (not startup or h % 2 == 0) else nc.scalar
                        eng.dma_start(
                            out=xt[:], in_=xT[b, h * 128:(h + 1) * 128, ssl])
                        xts.append(xt)
                    yield
                    for hp in range(2):
                        qps = ps_prj.tile([128, 512], F32, tag="prj",
                                          name=f"qp{b}{st}{hp}")
                        for h in range(NH):
                            nc.tensor.matmul(
                                qps[:], wq_t[h][:, hp * 128:(hp + 1) * 128],
                                xts[h][:],
                                start=(h == 0), stop=(h == NH - 1))
                            if h % 4 == 3:
                                yield
                        nc.vector.tensor_scalar_add(
                            qd2[hp][:, ssl], qps[:], bq_t[hp][:])
                        yield

            def pump(gen, n=1):
                if gen is None:
                    return None
                for _ in range(n):
                    try:
                        next(gen)
                    except StopIteration:
                        return None
                return gen

            def attn_phase(b, filler, pump_n=1):
                """Attention for batch b; pumps `filler` once per kc step."""
                st_ = state[b]
                kTd, v_aug, qd2 = st_["kTd"], st_["va"], st_["qd2"]
                oT_t = [
                    acts.tile([128, S], BF, tag=f"oT{hp}", name=f"oT{b}{hp}")
                    for hp in range(2)
                ]
                st_["oT"] = oT_t
                for hp in range(2):
                    av_sb = []
                    for qc in range(NQC):
                        qsl = slice(qc * 512, (qc + 1) * 512)
                        av0 = ps_av.tile([D + 1, 512], F32, tag="av",
                                         name=f"av0_{b}{hp}{qc}")
                        av1 = ps_av.tile([D + 1, 512], F32, tag="av",
                                         name=f"av1_{b}{hp}{qc}")
                        for kk in range(0, NKC, 2):
                            pair = (kk, kk + 1)
                            e2s = []
                            for kc in pair:
                                ksl = slice(kc * 128, (kc + 1) * 128)
                                sc2 = ps_sc.tile([128, 1024], F32, tag="sc",
                                                 name=f"sc{b}{hp}{qc}{kc}")
                                nc.tensor.matmul(
                                    sc2[:, 0:512], kTd[0:D, ksl],
                                    qd2[hp][0:D, qsl],
                                    start=True, stop=True, tile_position=(0, 0))
                                nc.tensor.matmul(
                                    sc2[:, 512:1024], kTd[D:128, ksl],
                                    qd2[hp][D:128, qsl],
                                    start=True, stop=True, tile_position=(64, 0))
                                e2 = epool.tile([128, 1024], BF, tag="e",
                                                name=f"e{b}{hp}{qc}{kc}")
                                nc.scalar.activation(
                                    e2[:], sc2[:], EXP, scale=float(SCALE))
                                e2s.append(e2)
                            for kc, e2 in zip(pair, e2s):
                                nc.tensor.matmul(
                                    av0[:], v_aug[kc][:], e2[:, 0:512],
                                    start=(kc == 0), stop=(kc == NKC - 1))
                                nc.tensor.matmul(
                                    av1[:], v_aug[kc][:], e2[:, 512:1024],
                                    start=(kc == 0), stop=(kc == NKC - 1))
                            filler = pump(filler, pump_n)
                        a0 = avsb_pool.tile([D + 1, 512], BF, tag="avsb",
                                            name=f"a0_{b}{hp}{qc}")
                        a1 = avsb_pool.tile([D + 1, 512], BF, tag="avsb",
                                            name=f"a1_{b}{hp}{qc}")
                        nc.vector.tensor_copy(a0[:], av0[:])
                        nc.vector.tensor_copy(a1[:], av1[:])
                        av_sb.append((qc, a0, a1))

                    # Z rows -> [128, 32] -> one reciprocal -> [1, 2048] rows
                    zP = zpool.tile([128, 32], BF, tag="zP", name=f"zP{b}{hp}")
                    for qc, a0, a1 in av_sb:
                        nc.sync.dma_start(
                            out=zP[:, qc * 4:(qc + 1) * 4], in_=a0[D:D + 1, :])
                        nc.sync.dma_start(
                            out=zP[:, 16 + qc * 4:16 + (qc + 1) * 4],
                            in_=a1[D:D + 1, :])
                    rP = zpool.tile([128, 32], BF, tag="rP", name=f"rP{b}{hp}")
                    with nc.allow_low_precision("bf16 1/Z broadcast"):
                        nc.vector.reciprocal(rP[:], zP[:])
                    rrow = [
                        zpool.tile([1, S], BF, tag="rrow", name=f"rr{b}{hp}{j}")
                        for j in range(2)
                    ]
                    for j in range(2):
                        for qc in range(NQC):
                            nc.sync.dma_start(
                                out=rrow[j][:, qc * 512:(qc + 1) * 512],
                                in_=rP[:, j * 16 + qc * 4:j * 16 + (qc + 1) * 4])
                    for qc, a0, a1 in av_sb:
                        qsl = slice(qc * 512, (qc + 1) * 512)
                        for j, av in ((0, a0), (1, a1)):
                            bc = ps_pr.tile([D, 512], F32, tag="pr",
                                            name=f"bc{b}{hp}{qc}{j}")
                            nc.tensor.matmul(
                                bc[:], ones1[:], rrow[j][:, qsl],
                                start=True, stop=True)
                            nc.vector.tensor_mul(
                                oT_t[hp][j * D:(j + 1) * D, qsl],
                                av[0:D, :], bc[:])
                        filler = pump(filler)
                # drain any remaining filler
                while filler is not None:
                    filler = pump(filler)

            def oproj_phase(b, final=False):
                """Generator: o-projection for batch b. In `final` mode (the
                un-overlapped tail) it borrows the wide sc PSUM slots and
                alternates casts between DVE and the now-idle ACT; in filler
                mode it drips through the pr/prj single-bank slots."""
                oT_t = state[b]["oT"]
                nprj = 0
                for sc16 in range(S // 128):
                    s128 = slice(sc16 * 128, (sc16 + 1) * 128)
                    ob = osb_pool.tile([128, HID], BF, tag="osb",
                                       name=f"ob{b}{sc16}")
                    if final:
                        for h2 in range(2):
                            ops = ps_sc.tile([128, 1024], F32, tag="sc",
                                             name=f"op{b}{sc16}{h2}")
                            for j in range(2):
                                hsl = slice((h2 * 2 + j) * 512,
                                            (h2 * 2 + j + 1) * 512)
                                osl = slice(j * 512, (j + 1) * 512)
                                nc.tensor.matmul(
                                    ops[:, osl], oT_t[0][:, s128],
                                    wo_t[0][:, hsl], start=True, stop=False)
                                nc.tensor.matmul(
                                    ops[:, osl], oT_t[1][:, s128],
                                    wo_t[1][:, hsl], start=False, stop=True)
                            dsl = slice(h2 * 1024, (h2 + 1) * 1024)
                            if h2 == 0:
                                nc.vector.tensor_copy(ob[:, dsl], ops[:])
                            else:
                                nc.scalar.copy(ob[:, dsl], ops[:])
                            yield
                    else:
                        for hc in range(HID // 512):
                            hsl = slice(hc * 512, (hc + 1) * 512)
                            tag = "pr" if nprj % 2 == 0 else "prj"
                            pool = ps_pr if nprj % 2 == 0 else ps_prj
                            nprj += 1
                            ops = pool.tile([128, 512], F32, tag=tag,
                                            name=f"op{b}{sc16}{hc}")
                            nc.tensor.matmul(
                                ops[:], oT_t[0][:, s128], wo_t[0][:, hsl],
                                start=True, stop=False)
                            nc.tensor.matmul(
                                ops[:], oT_t[1][:, s128], wo_t[1][:, hsl],
                                start=False, stop=True)
                            nc.vector.tensor_copy(ob[:, hsl], ops[:])
                            yield
                    nc.sync.dma_start(out=out[b, s128, :], in_=ob[:])
                    yield

            # ---- pipeline ----
            p0 = proj_phase(0, startup=True)
            while pump(p0) is not None:
                pass
            attn_phase(0, filler=proj_phase(1, startup=False), pump_n=2)
            o0 = oproj_phase(0)
            attn_phase(1, filler=o0, pump_n=2)
            o1 = oproj_phase(1, final=True)
            while pump(o1) is not None:
                pass

    if not nc.is_finalized():
        nc.finalize()
    return nc


_NC = None
_RUN_KWARGS = {}
_LAST_RESULT = None


def _get_nc():
    global _NC
    if _NC is None:
        _NC = _build_nc()
    return _NC


def kernel(x, encoder_output, Wq, bq, Wk, bk, Wv, bv, Wo, bo):
    nc = _get_nc()
    xT = np.ascontiguousarray(
        np.asarray(x, np.float32).transpose(0, 2, 1)).astype(BF16)
    encT = np.ascontiguousarray(
        np.asarray(encoder_output, np.float32).transpose(0, 2, 1)).astype(BF16)
    Wq = np.asarray(Wq, np.float32)
    Wk = np.asarray(Wk, np.float32)
    Wv = np.asarray(Wv, np.float32)
    Wo = np.asarray(Wo, np.float32)
    bq = np.asarray(bq, np.float32)
    bk = np.asarray(bk, np.float32)
    bv = np.asarray(bv, np.float32)
    in_maps = []
    for c in range(NCORES):
        csl = slice(c * CH, (c + 1) * CH)
        gsl = slice(c * D, (c + 1) * D)
        in_maps.append({
            "xT": xT,
            "encT": encT,
            "wq": np.ascontiguousarray(Wq[:, csl]).astype(BF16),
            "wkv": np.ascontiguousarray(
                np.concatenate([Wk[:, gsl], Wv[:, gsl]], axis=1)).astype(BF16),
            "wo": np.ascontiguousarray(Wo[csl, :]).astype(BF16),
            "bq": np.ascontiguousarray(bq[csl].reshape(CH, 1)),
            "bkv": np.ascontiguousarray(
                np.concatenate([bk[gsl], bv[gsl]]).reshape(128, 1)),
        })
    res = run_bass_kernel_spmd(nc, in_maps, list(range(NCORES)), **_RUN_KWARGS)
    global _LAST_RESULT
    _LAST_RESULT = res
    total = np.zeros((B, S, HID), np.float32)
    for c in range(NCORES):
        total += res.results[c]["out"].astype(np.float32)
    return total + np.asarray(bo, np.float32)


# revision 12
# speedup vs baseline: 1.0009x; 1.0009x over previous
"""GQA cross-attention block on 8 trn2 NeuronCores.

Sharding: tensor-parallel over heads. Core c owns KV group g=c (64 dims of
K/V) and its 4 query heads (256 q channels). Each core computes its heads'
attention plus its slice of the o-projection (rows c*256:(c+1)*256 of Wo),
producing a full-shape partial output; the host sums the 8 partials and
adds bo.

Structure (v3):
  - K/V projection packed: stationary [128h, 128] = [Wk_h | Wv_h] ->
    kvT [128, S] (K rows 0:64, V rows 64:128). Evacuated on DVE
    (tensor_scalar_add) so ACT stays reserved for exp.
  - Scores row-tiled 2x on the PE: kTd [128, S] holds K duplicated on both
    partition halves; qd2 [128, S] holds a HEAD PAIR. Two concurrent K=64
    matmuls (tile_position (0,0)/(64,0)) fill one [128, 1024] PSUM pair ->
    a single [128, 1024] exp on ACT (the kernel bottleneck: ~220us/core of
    pure exp streaming).
  - AV via v_aug [128, 65] (ones column -> softmax denominator Z free in
    row 64). Z rows batched into one [128, 32] reciprocal via DMA
    gather/scatter; 1/Z broadcast with a K=1 PE matmul; DVE mul -> oT.
  - Cross-batch software pipelining by interleaved EMISSION: batch 1's
    projection work is emitted in small chunks inside batch 0's attention
    kc-loop (and batch 0's o-projection inside batch 1's attention), so
    the Tile scheduler's priority order alternates and the PE fills the
    ACT-bound gaps. PSUM tags: sc 4 banks, av 2, prj 1, pr 1.
  - DMA spread: enc on gpsimd, x on sync(+scalar at startup), weights on
    scalar, z-dance + stores on sync.
"""

import numpy as np
import ml_dtypes

import concourse.bass as bass
from concourse import bacc
import concourse.mybir as mybir
import concourse.tile as tile
from concourse.bass_utils import run_bass_kernel_spmd
from concourse.masks import make_identity

BF16 = ml_dtypes.bfloat16
F32 = mybir.dt.float32
BF = mybir.dt.bfloat16

B = 2
S = 2048
HID = 2048
D = 64          # head dim
CH = 4 * D      # 256 q channels per core
NCORES = 8
NH = HID // 128  # 16 hidden chunks
NKC = S // 128   # 16 key chunks of 128
NQC = S // 512   # 4 q tiles of 512
NST = S // 512   # 4 s tiles of 512
SCALE = 1.0 / np.sqrt(D)


def _build_nc() -> bass.Bass:
    nc = bacc.Bacc()

    xT = nc.dram_tensor("xT", [B, HID, S], BF, kind="ExternalInput")
    encT = nc.dram_tensor("encT", [B, HID, S], BF, kind="ExternalInput")
    wq = nc.dram_tensor("wq", [HID, CH], BF, kind="ExternalInput")
    wkv = nc.dram_tensor("wkv", [HID, 128], BF, kind="ExternalInput")
    wo = nc.dram_tensor("wo", [CH, HID], BF, kind="ExternalInput")
    bq = nc.dram_tensor("bq", [CH, 1], F32, kind="ExternalInput")
    bkv = nc.dram_tensor("bkv", [128, 1], F32, kind="ExternalInput")
    out = nc.dram_tensor("out", [B, S, HID], BF, kind="ExternalOutput")

    EXP = mybir.ActivationFunctionType.Exp

    with tile.TileContext(nc) as tc:
        with (
            tc.tile_pool(name="wpool", bufs=1) as wpool,
            tc.tile_pool(name="io", bufs=22) as io_pool,
            tc.tile_pool(name="acts", bufs=2) as acts,
            tc.tile_pool(name="vaug", bufs=2 * NKC) as vaug_pool,
            tc.tile_pool(name="epool", bufs=3) as epool,
            tc.tile_pool(name="avsb", bufs=8) as avsb_pool,
            tc.tile_pool(name="zp", bufs=2) as zpool,
            tc.tile_pool(name="osb", bufs=2) as osb_pool,
            tc.tile_pool(name="ps_sc", bufs=2, space="PSUM") as ps_sc,
            tc.tile_pool(name="ps_av", bufs=2, space="PSUM") as ps_av,
            tc.tile_pool(name="ps_prj", bufs=1, space="PSUM") as ps_prj,
            tc.tile_pool(name="ps_pr", bufs=1, space="PSUM") as ps_pr,
        ):
            # ---- resident weights / constants (scalar HWDGE queue) ----
            wkv_t = []
            for h in range(NH):
                wkvh = wpool.tile([128, 128], BF, name=f"wkv{h}")
                nc.scalar.dma_start(out=wkvh[:], in_=wkv[h * 128:(h + 1) * 128, :])
                wkv_t.append(wkvh)
            bkv_t = wpool.tile([128, 1], F32, name="bkv_t")
            nc.scalar.dma_start(out=bkv_t[:], in_=bkv[:, :])
            ident = wpool.tile([128, 128], BF, name="ident")
            make_identity(nc, ident[:])
            wq_t = []
            for h in range(NH):
                wqh = wpool.tile([128, CH], BF, name=f"wq{h}")
                nc.scalar.dma_start(out=wqh[:], in_=wq[h * 128:(h + 1) * 128, :])
                wq_t.append(wqh)
            bq_t = []
            for cc in range(2):
                bqc = wpool.tile([128, 1], F32, name=f"bq{cc}")
                nc.scalar.dma_start(out=bqc[:], in_=bq[cc * 128:(cc + 1) * 128, :])
                bq_t.append(bqc)
            wo_t = []
            for cc in range(2):
                woc = wpool.tile([128, HID], BF, name=f"wo{cc}")
                nc.scalar.dma_start(out=woc[:], in_=wo[cc * 128:(cc + 1) * 128, :])
                wo_t.append(woc)
            ones1 = wpool.tile([1, D], BF, name="ones1")
            nc.gpsimd.memset(ones1[:], 1.0)

            state = {}

            def proj_phase(b, startup):
                """Generator: KV proj, kTd/vT dup, v_aug, Q proj for batch b.
                Yields between small chunks so it can be pumped as PE filler
                inside the other batch's attention loop."""
                st_ = {}
                state[b] = st_
                # --- KV projection ---
                kvT = acts.tile([128, S], BF, tag="kvT", bufs=1, name=f"kvT{b}")
                for st in range(NST):
                    ssl = slice(st * 512, (st + 1) * 512)
                    ets = []
                    for h in range(NH):
                        et = io_pool.tile([128, 512], BF, tag="io",
                                          name=f"es{b}{st}{h}")
                        eng = nc.gpsimd if h % 2 == 0 else (nc.sync if startup else nc.gpsimd)
                        eng.dma_start(
                            out=et[:], in_=encT[b, h * 128:(h + 1) * 128, ssl])
                        ets.append(et)
                    yield
                    kvps = ps_prj.tile([128, 512], F32, tag="prj",
                                       name=f"kvp{b}{st}")
                    for h in range(NH):
                        nc.tensor.matmul(
                            kvps[:], wkv_t[h][:], ets[h][:],
                            start=(h == 0), stop=(h == NH - 1))
                        if h % 4 == 3:
                            yield
                    nc.vector.tensor_scalar_add(kvT[:, ssl], kvps[:], bkv_t[:])
                    yield
                # --- kTd (K duplicated on both halves), vT ---
                kTd = acts.tile([128, S], BF, tag="kTd", name=f"kTd{b}")
                vT = acts.tile([D, S], BF, tag="vT", bufs=1, name=f"vT{b}")
                nc.gpsimd.dma_start(out=kTd[0:D, :], in_=kvT[0:D, :])
                nc.gpsimd.dma_start(out=kTd[D:128, :], in_=kvT[0:D, :])
                nc.gpsimd.dma_start(out=vT[:], in_=kvT[D:128, :])
                st_["kTd"] = kTd
                yield
                # --- v_aug chunks [128, 65] with ones in col 64 ---
                v_aug = []
                for kc in range(NKC):
                    vtp = ps_pr.tile([128, D], BF, tag="pr", name=f"vtp{b}{kc}")
                    nc.tensor.transpose(
                        vtp[:], vT[:, kc * 128:(kc + 1) * 128], ident[0:D, 0:D])
                    va = vaug_pool.tile([128, D + 1], BF, tag=f"va{kc}",
                                        name=f"va{b}{kc}")
                    nc.gpsimd.memset(va[:, D:D + 1], 1.0)
                    nc.vector.tensor_copy(va[:, 0:D], vtp[:])
                    v_aug.append(va)
                    if kc % 4 == 3:
                        yield
                st_["va"] = v_aug
                # --- Q projection -> head-pair tiles qd2[hp] ---
                qd2 = [
                    acts.tile([128, S], BF, tag=f"qd{hp}", name=f"qd{b}{hp}")
                    for hp in range(2)
                ]
                st_["qd2"] = qd2
                for st in range(NST):
                    ssl = slice(st * 512, (st + 1) * 512)
                    xts = []
                    for h in range(NH):
                        xt = io_pool.tile([128, 512], BF, tag="io",
                                          name=f"xs{b}{st}{h}")
                        eng = nc.sync if (not startup or h % 2 == 0) else nc.scalar
                        eng.dma_start(
                            out=xt[:], in_=xT[b, h * 128:(h + 1) * 128, ssl])
                        xts.append(xt)
                    yield
                    for hp in range(2):
                        qps = ps_prj.tile([128, 512], F32, tag="prj",
                                          name=f"qp{b}{st}{hp}")
                        for h in range(NH):
                            nc.tensor.matmul(
                                qps[:], wq_t[h][:, hp * 128:(hp + 1) * 128],
                                xts[h][:],
                                start=(h == 0), stop=(h == NH - 1))
                            if h % 4 == 3:
                                yield
                        nc.vector.tensor_scalar_add(
                            qd2[hp][:, ssl], qps[:], bq_t[hp][:])
                        yield

            def pump(gen, n=1):
                if gen is None:
                    return None
                for _ in range(n):
                    try:
                        next(gen)
                    except StopIteration:
                        return None
                return gen

            def attn_phase(b, filler, pump_n=1):
                """Attention for batch b; pumps `filler` once per kc step."""
                st_ = state[b]
                kTd, v_aug, qd2 = st_["kTd"], st_["va"], st_["qd2"]
                oT_t = [
                    acts.tile([128, S], BF, tag=f"oT{hp}", name=f"oT{b}{hp}")
                    for hp in range(2)
                ]
                st_["oT"] = oT_t
                for hp in range(2):
                    av_sb = []
                    for qc in range(NQC):
                        qsl = slice(qc * 512, (qc + 1) * 512)
                        av0 = ps_av.tile([D + 1, 512], F32, tag="av",
                                         name=f"av0_{b}{hp}{qc}")
                        av1 = ps_av.tile([D + 1, 512], F32, tag="av",
                                         name=f"av1_{b}{hp}{qc}")
                        # scores+exp run one kc ahead of AV so the PE FIFO
                        # never head-of-line blocks on exp (PE reorders only
                        # LDWEIGHTS, not MATMULs).
                        e_ahead = {}

                        def emit_scores(kc):
                            ksl = slice(kc * 128, (kc + 1) * 128)
                            sc2 = ps_sc.tile([128, 1024], F32, tag="sc",
                                             name=f"sc{b}{hp}{qc}{kc}")
                            nc.tensor.matmul(
                                sc2[:, 0:512], kTd[0:D, ksl],
                                qd2[hp][0:D, qsl],
                                start=True, stop=True, tile_position=(0, 0))
                            nc.tensor.matmul(
                                sc2[:, 512:1024], kTd[D:128, ksl],
                                qd2[hp][D:128, qsl],
                                start=True, stop=True, tile_position=(64, 0))
                            e2 = epool.tile([128, 1024], BF, tag="e",
                                            name=f"e{b}{hp}{qc}{kc}")
                            nc.scalar.activation(
                                e2[:], sc2[:], EXP, scale=float(SCALE))
                            e_ahead[kc] = e2

                        emit_scores(0)
                        for kc in range(NKC):
                            if kc + 1 < NKC:
                                emit_scores(kc + 1)
                            e2 = e_ahead.pop(kc)
                            nc.tensor.matmul(
                                av0[:], v_aug[kc][:], e2[:, 0:512],
                                start=(kc == 0), stop=(kc == NKC - 1))
                            nc.tensor.matmul(
                                av1[:], v_aug[kc][:], e2[:, 512:1024],
                                start=(kc == 0), stop=(kc == NKC - 1))
                            filler = pump(filler, pump_n)
                        a0 = avsb_pool.tile([D + 1, 512], BF, tag="avsb",
                                            name=f"a0_{b}{hp}{qc}")
                        a1 = avsb_pool.tile([D + 1, 512], BF, tag="avsb",
                                            name=f"a1_{b}{hp}{qc}")
                        nc.vector.tensor_copy(a0[:], av0[:])
                        nc.vector.tensor_copy(a1[:], av1[:])
                        av_sb.append((qc, a0, a1))

                    # Z rows -> [128, 32] -> one reciprocal -> [1, 2048] rows
                    zP = zpool.tile([128, 32], BF, tag="zP", name=f"zP{b}{hp}")
                    for qc, a0, a1 in av_sb:
                        nc.sync.dma_start(
                            out=zP[:, qc * 4:(qc + 1) * 4], in_=a0[D:D + 1, :])
                        nc.sync.dma_start(
                            out=zP[:, 16 + qc * 4:16 + (qc + 1) * 4],
                            in_=a1[D:D + 1, :])
                    rP = zpool.tile([128, 32], BF, tag="rP", name=f"rP{b}{hp}")
                    with nc.allow_low_precision("bf16 1/Z broadcast"):
                        nc.vector.reciprocal(rP[:], zP[:])
                    rrow = [
                        zpool.tile([1, S], BF, tag="rrow", name=f"rr{b}{hp}{j}")
                        for j in range(2)
                    ]
                    for j in range(2):
                        for qc in range(NQC):
                            nc.sync.dma_start(
                                out=rrow[j][:, qc * 512:(qc + 1) * 512],
                                in_=rP[:, j * 16 + qc * 4:j * 16 + (qc + 1) * 4])
                    for qc, a0, a1 in av_sb:
                        qsl = slice(qc * 512, (qc + 1) * 512)
                        for j, av in ((0, a0), (1, a1)):
                            bc = ps_pr.tile([D, 512], F32, tag="pr",
                                            name=f"bc{b}{hp}{qc}{j}")
                            nc.tensor.matmul(
                                bc[:], ones1[:], rrow[j][:, qsl],
                                start=True, stop=True)
                            nc.vector.tensor_mul(
                                oT_t[hp][j * D:(j + 1) * D, qsl],
                                av[0:D, :], bc[:])
                        filler = pump(filler)
                # drain any remaining filler
                while filler is not None:
                    filler = pump(filler)

            def oproj_phase(b, final=False):
                """Generator: o-projection for batch b. In `final` mode (the
                un-overlapped tail) it borrows the wide sc PSUM slots and
                alternates casts between DVE and the now-idle ACT; in filler
                mode it drips through the pr/prj single-bank slots."""
                oT_t = state[b]["oT"]
                nprj = 0
                for sc16 in range(S // 128):
                    s128 = slice(sc16 * 128, (sc16 + 1) * 128)
                    ob = osb_pool.tile([128, HID], BF, tag="osb",
                                       name=f"ob{b}{sc16}")
                    if final:
                        for h2 in range(2):
                            ops = ps_sc.tile([128, 1024], F32, tag="sc",
                                             name=f"op{b}{sc16}{h2}")
                            for j in range(2):
                                hsl = slice((h2 * 2 + j) * 512,
                                            (h2 * 2 + j + 1) * 512)
                                osl = slice(j * 512, (j + 1) * 512)
                                nc.tensor.matmul(
                                    ops[:, osl], oT_t[0][:, s128],
                                    wo_t[0][:, hsl], start=True, stop=False)
                                nc.tensor.matmul(
                                    ops[:, osl], oT_t[1][:, s128],
                                    wo_t[1][:, hsl], start=False, stop=True)
                            dsl = slice(h2 * 1024, (h2 + 1) * 1024)
                            if h2 == 0:
                                nc.vector.tensor_copy(ob[:, dsl], ops[:])
                            else:
                                nc.scalar.copy(ob[:, dsl], ops[:])
                            yield
                    else:
                        for hc in range(HID // 512):
                            hsl = slice(hc * 512, (hc + 1) * 512)
                            tag = "pr" if nprj % 2 == 0 else "prj"
                            pool = ps_pr if nprj % 2 == 0 else ps_prj
                            nprj += 1
                            ops = pool.tile([128, 512], F32, tag=tag,
                                            name=f"op{b}{sc16}{hc}")
                            nc.tensor.matmul(
                                ops[:], oT_t[0][:, s128], wo_t[0][:, hsl],
                                start=True, stop=False)
                            nc.tensor.matmul(
                                ops[:], oT_t[1][:, s128], wo_t[1][:, hsl],
                                start=False, stop=True)
                            nc.vector.tensor_copy(ob[:, hsl], ops[:])
                            yield
                    nc.sync.dma_start(out=out[b, s128, :], in_=ob[:])
                    yield

            # ---- pipeline ----
            p0 = proj_phase(0, startup=True)
            while pump(p0) is not None:
                pass
            attn_phase(0, filler=proj_phase(1, startup=False), pump_n=1)
            o0 = oproj_phase(0)
            attn_phase(1, filler=o0, pump_n=1)
            o1 = oproj_phase(1, final=True)
            while pump(o1) is not None:
                pass

    if not nc.is_finalized():
        nc.finalize()
    return nc


_NC = None
_RUN_KWARGS = {}
_LAST_RESULT = None


def _get_nc():
    global _NC
    if _NC is None:
        _NC = _build_nc()
    return _NC


def kernel(x, encoder_output, Wq, bq, Wk, bk, Wv, bv, Wo, bo):
    nc = _get_nc()
    xT = np.ascontiguousarray(
        np.asarray(x, np.float32).transpose(0, 2, 1)).astype(BF16)
    encT = np.ascontiguousarray(
        np.asarray(encoder_output, np.float32).transpose(0, 2, 1)).astype(BF16)
    Wq = np.asarray(Wq, np.float32)
    Wk = np.asarray(Wk, np.float32)
    Wv = np.asarray(Wv, np.float32)
    Wo = np.asarray(Wo, np.float32)
    bq = np.asarray(bq, np.float32)
    bk = np.asarray(bk, np.float32)
    bv = np.asarray(bv, np.float32)
    in_maps = []
    for c in range(NCORES):
        csl = slice(c * CH, (c + 1) * CH)
        gsl = slice(c * D, (c + 1) * D)
        in_maps.append({
            "xT": xT,
            "encT": encT,
            "wq": np.ascontiguousarray(Wq[:, csl]).astype(BF16),
            "wkv": np.ascontiguousarray(
                np.concatenate([Wk[:, gsl], Wv[:, gsl]], axis=1)).astype(BF16),
            "wo": np.ascontiguousarray(Wo[csl, :]).astype(BF16),
            "bq": np.ascontiguousarray(bq[csl].reshape(CH, 1)),
            "bkv": np.ascontiguousarray(
                np.concatenate([bk[gsl], bv[gsl]]).reshape(128, 1)),
        })
    res = run_bass_kernel_spmd(nc, in_maps, list(range(NCORES)), **_RUN_KWARGS)
    global _LAST_RESULT
    _LAST_RESULT = res
    total = np.zeros((B, S, HID), np.float32)
    for c in range(NCORES):
        total += res.results[c]["out"].astype(np.float32)
    return total + np.asarray(bo, np.float32)


# revision 13
# speedup vs baseline: 1.1889x; 1.1877x over previous
"""GQA cross-attention block on 8 trn2 NeuronCores.

Sharding: tensor-parallel over heads. Core c owns KV group g=c (64 dims of
K/V) and its 4 query heads (256 q channels). Each core computes its heads'
attention plus its slice of the o-projection (rows c*256:(c+1)*256 of Wo),
producing a full-shape partial output; the host sums the 8 partials and
adds bo.

Structure (v3):
  - K/V projection packed: stationary [128h, 128] = [Wk_h | Wv_h] ->
    kvT [128, S] (K rows 0:64, V rows 64:128). Evacuated on DVE
    (tensor_scalar_add) so ACT stays reserved for exp.
  - Scores row-tiled 2x on the PE: kTd [128, S] holds K duplicated on both
    partition halves; qd2 [128, S] holds a HEAD PAIR. Two concurrent K=64
    matmuls (tile_position (0,0)/(64,0)) fill one [128, 1024] PSUM pair ->
    a single [128, 1024] exp on ACT (the kernel bottleneck: ~220us/core of
    pure exp streaming).
  - AV via v_aug [128, 65] (ones column -> softmax denominator Z free in
    row 64). Z rows batched into one [128, 32] reciprocal via DMA
    gather/scatter; 1/Z broadcast with a K=1 PE matmul; DVE mul -> oT.
  - Cross-batch software pipelining by interleaved EMISSION: batch 1's
    projection work is emitted in small chunks inside batch 0's attention
    kc-loop (and batch 0's o-projection inside batch 1's attention), so
    the Tile scheduler's priority order alternates and the PE fills the
    ACT-bound gaps. PSUM tags: sc 4 banks, av 2, prj 1, pr 1.
  - DMA spread: enc on gpsimd, x on sync(+scalar at startup), weights on
    scalar, z-dance + stores on sync.
"""

import numpy as np
import ml_dtypes

import concourse.bass as bass
from concourse import bacc
import concourse.mybir as mybir
import concourse.tile as tile
from concourse.bass_utils import run_bass_kernel_spmd
from concourse.masks import make_identity

BF16 = ml_dtypes.bfloat16
F32 = mybir.dt.float32
BF = mybir.dt.bfloat16

B = 2
S = 2048
HID = 2048
D = 64          # head dim
CH = 4 * D      # 256 q channels per core
NCORES = 8
NH = HID // 128  # 16 hidden chunks
NKC = S // 128   # 16 key chunks of 128
NQC = S // 512   # 4 q tiles of 512
NST = S // 512   # 4 s tiles of 512
SCALE = 1.0 / np.sqrt(D)


def _build_nc() -> bass.Bass:
    nc = bacc.Bacc()

    xT = nc.dram_tensor("xT", [B, HID, S], BF, kind="ExternalInput")
    encT = nc.dram_tensor("encT", [B, HID, S], BF, kind="ExternalInput")
    wq = nc.dram_tensor("wq", [HID, CH], BF, kind="ExternalInput")
    wkv = nc.dram_tensor("wkv", [HID, 128], BF, kind="ExternalInput")
    wo = nc.dram_tensor("wo", [CH, HID], BF, kind="ExternalInput")
    bq = nc.dram_tensor("bq", [CH, 1], F32, kind="ExternalInput")
    bkv = nc.dram_tensor("bkv", [128, 1], F32, kind="ExternalInput")
    out = nc.dram_tensor("out", [B, S, HID], BF, kind="ExternalOutput")

    EXP = mybir.ActivationFunctionType.Exp

    with tile.TileContext(nc) as tc:
        with (
            tc.tile_pool(name="wpool", bufs=1) as wpool,
            tc.tile_pool(name="io", bufs=22) as io_pool,
            tc.tile_pool(name="acts", bufs=2) as acts,
            tc.tile_pool(name="vaug", bufs=2 * NKC) as vaug_pool,
            tc.tile_pool(name="epool", bufs=3) as epool,
            tc.tile_pool(name="avsb", bufs=8) as avsb_pool,
            tc.tile_pool(name="zp", bufs=2) as zpool,
            tc.tile_pool(name="osb", bufs=2) as osb_pool,
            tc.tile_pool(name="ps_sc", bufs=2, space="PSUM") as ps_sc,
            tc.tile_pool(name="ps_av", bufs=2, space="PSUM") as ps_av,
            tc.tile_pool(name="ps_prj", bufs=1, space="PSUM") as ps_prj,
            tc.tile_pool(name="ps_pr", bufs=1, space="PSUM") as ps_pr,
        ):
            # ---- resident weights / constants (scalar HWDGE queue) ----
            wkv_t = []
            for h in range(NH):
                wkvh = wpool.tile([128, 128], BF, name=f"wkv{h}")
                nc.scalar.dma_start(out=wkvh[:], in_=wkv[h * 128:(h + 1) * 128, :])
                wkv_t.append(wkvh)
            bkv_t = wpool.tile([128, 1], F32, name="bkv_t")
            nc.scalar.dma_start(out=bkv_t[:], in_=bkv[:, :])
            ident = wpool.tile([128, 128], BF, name="ident")
            make_identity(nc, ident[:])
            wq_t = []
            for h in range(NH):
                wqh = wpool.tile([128, CH], BF, name=f"wq{h}")
                nc.scalar.dma_start(out=wqh[:], in_=wq[h * 128:(h + 1) * 128, :])
                wq_t.append(wqh)
            bq_t = []
            for cc in range(2):
                bqc = wpool.tile([128, 1], F32, name=f"bq{cc}")
                nc.scalar.dma_start(out=bqc[:], in_=bq[cc * 128:(cc + 1) * 128, :])
                bq_t.append(bqc)
            wo_t = []
            for cc in range(2):
                woc = wpool.tile([128, HID], BF, name=f"wo{cc}")
                nc.scalar.dma_start(out=woc[:], in_=wo[cc * 128:(cc + 1) * 128, :])
                wo_t.append(woc)
            ones1 = wpool.tile([1, D], BF, name="ones1")
            nc.gpsimd.memset(ones1[:], 1.0)

            state = {}

            def proj_phase(b, startup):
                """Generator: KV proj, kTd/vT dup, v_aug, Q proj for batch b.
                Yields between small chunks so it can be pumped as PE filler
                inside the other batch's attention loop."""
                st_ = {}
                state[b] = st_
                # --- KV projection ---
                kvT = acts.tile([128, S], BF, tag="kvT", bufs=1, name=f"kvT{b}")
                for st in range(NST):
                    ssl = slice(st * 512, (st + 1) * 512)
                    ets = []
                    for h in range(NH):
                        et = io_pool.tile([128, 512], BF, tag="io",
                                          name=f"es{b}{st}{h}")
                        eng = nc.gpsimd if h % 2 == 0 else (nc.sync if startup else nc.gpsimd)
                        eng.dma_start(
                            out=et[:], in_=encT[b, h * 128:(h + 1) * 128, ssl])
                        ets.append(et)
                    yield
                    kvps = ps_prj.tile([128, 512], F32, tag="prj",
                                       name=f"kvp{b}{st}")
                    for h in range(NH):
                        nc.tensor.matmul(
                            kvps[:], wkv_t[h][:], ets[h][:],
                            start=(h == 0), stop=(h == NH - 1))
                        if h % 4 == 3:
                            yield
                    nc.vector.tensor_scalar_add(kvT[:, ssl], kvps[:], bkv_t[:])
                    yield
                # --- kTd (K duplicated on both halves), vT ---
                kTd = acts.tile([128, S], BF, tag="kTd", name=f"kTd{b}")
                vT = acts.tile([D, S], BF, tag="vT", bufs=1, name=f"vT{b}")
                nc.gpsimd.dma_start(out=kTd[0:D, :], in_=kvT[0:D, :])
                nc.gpsimd.dma_start(out=kTd[D:128, :], in_=kvT[0:D, :])
                nc.gpsimd.dma_start(out=vT[:], in_=kvT[D:128, :])
                st_["kTd"] = kTd
                yield
                # --- v_aug chunks [128, 65] with ones in col 64 ---
                v_aug = []
                for kc in range(NKC):
                    vtp = ps_pr.tile([128, D], BF, tag="pr", name=f"vtp{b}{kc}")
                    nc.tensor.transpose(
                        vtp[:], vT[:, kc * 128:(kc + 1) * 128], ident[0:D, 0:D])
                    va = vaug_pool.tile([128, D + 1], BF, tag=f"va{kc}",
                                        name=f"va{b}{kc}")
                    nc.gpsimd.memset(va[:, D:D + 1], 1.0)
                    nc.vector.tensor_copy(va[:, 0:D], vtp[:])
                    v_aug.append(va)
                    if kc % 4 == 3:
                        yield
                st_["va"] = v_aug
                # --- Q projection -> head-pair tiles qd2[hp] ---
                qd2 = [
                    acts.tile([128, S], BF, tag=f"qd{hp}", name=f"qd{b}{hp}")
                    for hp in range(2)
                ]
                st_["qd2"] = qd2
                for st in range(NST):
                    ssl = slice(st * 512, (st + 1) * 512)
                    xts = []
                    for h in range(NH):
                        xt = io_pool.tile([128, 512], BF, tag="io",
                                          name=f"xs{b}{st}{h}")
                        eng = nc.sync if (not startup or h % 2 == 0) else nc.scalar
                        eng.dma_start(
                            out=xt[:], in_=xT[b, h * 128:(h + 1) * 128, ssl])
                        xts.append(xt)
                    yield
                    for hp in range(2):
                        qps = ps_prj.tile([128, 512], F32, tag="prj",
                                          name=f"qp{b}{st}{hp}")
                        for h in range(NH):
                            nc.tensor.matmul(
                                qps[:], wq_t[h][:, hp * 128:(hp + 1) * 128],
                                xts[h][:],
                                start=(h == 0), stop=(h == NH - 1))
                            if h % 4 == 3:
                                yield
                        nc.vector.tensor_scalar_add(
                            qd2[hp][:, ssl], qps[:], bq_t[hp][:])
                        yield

            def pump(gen, n=1):
                if gen is None:
                    return None
                for _ in range(n):
                    try:
                        next(gen)
                    except StopIteration:
                        return None
                return gen

            def attn_phase(b, filler, pump_n=1):
                """Attention for batch b; pumps `filler` once per kc step."""
                st_ = state[b]
                kTd, v_aug, qd2 = st_["kTd"], st_["va"], st_["qd2"]
                oT_t = [
                    acts.tile([128, S], BF, tag=f"oT{hp}", name=f"oT{b}{hp}")
                    for hp in range(2)
                ]
                st_["oT"] = oT_t
                def norm_gen(av_sb, rrow, oT):
                    # deferred 1/Z broadcast + multiply: pumped inside the
                    # NEXT section's kc loop so the recip latency chain never
                    # head-of-line blocks the PE FIFO.
                    for qc, a0, a1 in av_sb:
                        qsl = slice(qc * 512, (qc + 1) * 512)
                        for j, av in ((0, a0), (1, a1)):
                            bc = ps_pr.tile([D, 512], F32, tag="pr",
                                            name=f"bc{b}{id(a0)}{j}")
                            nc.tensor.matmul(
                                bc[:], ones1[:], rrow[j][:, qsl],
                                start=True, stop=True)
                            nc.vector.tensor_mul(
                                oT[j * D:(j + 1) * D, qsl], av[0:D, :], bc[:])
                        yield

                pending = None
                for hp in range(2):
                    av_sb = []
                    zP = zpool.tile([128, 32], BF, tag="zP", name=f"zP{b}{hp}")
                    for qc in range(NQC):
                        qsl = slice(qc * 512, (qc + 1) * 512)
                        av0 = ps_av.tile([D + 1, 512], F32, tag="av",
                                         name=f"av0_{b}{hp}{qc}")
                        av1 = ps_av.tile([D + 1, 512], F32, tag="av",
                                         name=f"av1_{b}{hp}{qc}")
                        # scores+exp run one kc ahead of AV so the PE FIFO
                        # never head-of-line blocks on exp (PE reorders only
                        # LDWEIGHTS, not MATMULs).
                        e_ahead = {}

                        def emit_scores(kc):
                            ksl = slice(kc * 128, (kc + 1) * 128)
                            sc2 = ps_sc.tile([128, 1024], F32, tag="sc",
                                             name=f"sc{b}{hp}{qc}{kc}")
                            nc.tensor.matmul(
                                sc2[:, 0:512], kTd[0:D, ksl],
                                qd2[hp][0:D, qsl],
                                start=True, stop=True, tile_position=(0, 0))
                            nc.tensor.matmul(
                                sc2[:, 512:1024], kTd[D:128, ksl],
                                qd2[hp][D:128, qsl],
                                start=True, stop=True, tile_position=(64, 0))
                            e2 = epool.tile([128, 1024], BF, tag="e",
                                            name=f"e{b}{hp}{qc}{kc}")
                            nc.scalar.activation(
                                e2[:], sc2[:], EXP, scale=float(SCALE))
                            e_ahead[kc] = e2

                        emit_scores(0)
                        for kc in range(NKC):
                            if kc + 1 < NKC:
                                emit_scores(kc + 1)
                            e2 = e_ahead.pop(kc)
                            nc.tensor.matmul(
                                av0[:], v_aug[kc][:], e2[:, 0:512],
                                start=(kc == 0), stop=(kc == NKC - 1))
                            nc.tensor.matmul(
                                av1[:], v_aug[kc][:], e2[:, 512:1024],
                                start=(kc == 0), stop=(kc == NKC - 1))
                            if pending is not None:
                                pending = pump(pending)
                            else:
                                filler = pump(filler, pump_n)
                        a0 = avsb_pool.tile([D + 1, 512], BF, tag="avsb",
                                            name=f"a0_{b}{hp}{qc}")
                        a1 = avsb_pool.tile([D + 1, 512], BF, tag="avsb",
                                            name=f"a1_{b}{hp}{qc}")
                        nc.vector.tensor_copy(a0[:], av0[:])
                        nc.vector.tensor_copy(a1[:], av1[:])
                        nc.sync.dma_start(
                            out=zP[:, qc * 4:(qc + 1) * 4], in_=a0[D:D + 1, :])
                        nc.sync.dma_start(
                            out=zP[:, 16 + qc * 4:16 + (qc + 1) * 4],
                            in_=a1[D:D + 1, :])
                        av_sb.append((qc, a0, a1))

                    # one batched reciprocal -> [1, 2048] rows
                    rP = zpool.tile([128, 32], BF, tag="rP", name=f"rP{b}{hp}")
                    with nc.allow_low_precision("bf16 1/Z broadcast"):
                        nc.vector.reciprocal(rP[:], zP[:])
                    rrow = [
                        zpool.tile([1, S], BF, tag="rrow", name=f"rr{b}{hp}{j}")
                        for j in range(2)
                    ]
                    for j in range(2):
                        for qc in range(NQC):
                            nc.sync.dma_start(
                                out=rrow[j][:, qc * 512:(qc + 1) * 512],
                                in_=rP[:, j * 16 + qc * 4:j * 16 + (qc + 1) * 4])
                    pending = norm_gen(av_sb, rrow, oT_t[hp])
                # drain the last head-pair's normalization, then filler
                while pending is not None:
                    pending = pump(pending)
                while filler is not None:
                    filler = pump(filler)

            def oproj_phase(b, final=False):
                """Generator: o-projection for batch b. In `final` mode (the
                un-overlapped tail) it borrows the wide sc PSUM slots and
                alternates casts between DVE and the now-idle ACT; in filler
                mode it drips through the pr/prj single-bank slots."""
                oT_t = state[b]["oT"]
                nprj = 0
                for sc16 in range(S // 128):
                    s128 = slice(sc16 * 128, (sc16 + 1) * 128)
                    ob = osb_pool.tile([128, HID], BF, tag="osb",
                                       name=f"ob{b}{sc16}")
                    if final:
                        for h2 in range(2):
                            ops = ps_sc.tile([128, 1024], F32, tag="sc",
                                             name=f"op{b}{sc16}{h2}")
                            for j in range(2):
                                hsl = slice((h2 * 2 + j) * 512,
                                            (h2 * 2 + j + 1) * 512)
                                osl = slice(j * 512, (j + 1) * 512)
                                nc.tensor.matmul(
                                    ops[:, osl], oT_t[0][:, s128],
                                    wo_t[0][:, hsl], start=True, stop=False)
                                nc.tensor.matmul(
                                    ops[:, osl], oT_t[1][:, s128],
                                    wo_t[1][:, hsl], start=False, stop=True)
                            dsl = slice(h2 * 1024, (h2 + 1) * 1024)
                            if h2 == 0:
                                nc.vector.tensor_copy(ob[:, dsl], ops[:])
                            else:
                                nc.scalar.copy(ob[:, dsl], ops[:])
                            yield
                    else:
                        for hc in range(HID // 512):
                            hsl = slice(hc * 512, (hc + 1) * 512)
                            tag = "pr" if nprj % 2 == 0 else "prj"
                            pool = ps_pr if nprj % 2 == 0 else ps_prj
                            nprj += 1
                            ops = pool.tile([128, 512], F32, tag=tag,
                                            name=f"op{b}{sc16}{hc}")
                            nc.tensor.matmul(
                                ops[:], oT_t[0][:, s128], wo_t[0][:, hsl],
                                start=True, stop=False)
                            nc.tensor.matmul(
                                ops[:], oT_t[1][:, s128], wo_t[1][:, hsl],
                                start=False, stop=True)
                            nc.vector.tensor_copy(ob[:, hsl], ops[:])
                            yield
                    nc.sync.dma_start(out=out[b, s128, :], in_=ob[:])
                    yield

            # ---- pipeline ----
            p0 = proj_phase(0, startup=True)
            while pump(p0) is not None:
                pass
            attn_phase(0, filler=proj_phase(1, startup=False), pump_n=1)
            o0 = oproj_phase(0)
            attn_phase(1, filler=o0, pump_n=1)
            o1 = oproj_phase(1, final=True)
            while pump(o1) is not None:
                pass

    if not nc.is_finalized():
        nc.finalize()
    return nc


_NC = None
_RUN_KWARGS = {}
_LAST_RESULT = None


def _get_nc():
    global _NC
    if _NC is None:
        _NC = _build_nc()
    return _NC


def kernel(x, encoder_output, Wq, bq, Wk, bk, Wv, bv, Wo, bo):
    nc = _get_nc()
    xT = np.ascontiguousarray(
        np.asarray(x, np.float32).transpose(0, 2, 1)).astype(BF16)
    encT = np.ascontiguousarray(
        np.asarray(encoder_output, np.float32).transpose(0, 2, 1)).astype(BF16)
    Wq = np.asarray(Wq, np.float32)
    Wk = np.asarray(Wk, np.float32)
    Wv = np.asarray(Wv, np.float32)
    Wo = np.asarray(Wo, np.float32)
    bq = np.asarray(bq, np.float32)
    bk = np.asarray(bk, np.float32)
    bv = np.asarray(bv, np.float32)
    in_maps = []
    for c in range(NCORES):
        csl = slice(c * CH, (c + 1) * CH)
        gsl = slice(c * D, (c + 1) * D)
        in_maps.append({
            "xT": xT,
            "encT": encT,
            "wq": np.ascontiguousarray(Wq[:, csl]).astype(BF16),
            "wkv": np.ascontiguousarray(
                np.concatenate([Wk[:, gsl], Wv[:, gsl]], axis=1)).astype(BF16),
            "wo": np.ascontiguousarray(Wo[csl, :]).astype(BF16),
            "bq": np.ascontiguousarray(bq[csl].reshape(CH, 1)),
            "bkv": np.ascontiguousarray(
                np.concatenate([bk[gsl], bv[gsl]]).reshape(128, 1)),
        })
    res = run_bass_kernel_spmd(nc, in_maps, list(range(NCORES)), **_RUN_KWARGS)
    global _LAST_RESULT
    _LAST_RESULT = res
    total = np.zeros((B, S, HID), np.float32)
    for c in range(NCORES):
        total += res.results[c]["out"].astype(np.float32)
    return total + np.asarray(bo, np.float32)


# revision 14
# speedup vs baseline: 1.2068x; 1.0151x over previous
"""GQA cross-attention block on 8 trn2 NeuronCores.

Sharding: tensor-parallel over heads. Core c owns KV group g=c (64 dims of
K/V) and its 4 query heads (256 q channels). Each core computes its heads'
attention plus its slice of the o-projection (rows c*256:(c+1)*256 of Wo),
producing a full-shape partial output; the host sums the 8 partials and
adds bo.

Structure (v3):
  - K/V projection packed: stationary [128h, 128] = [Wk_h | Wv_h] ->
    kvT [128, S] (K rows 0:64, V rows 64:128). Evacuated on DVE
    (tensor_scalar_add) so ACT stays reserved for exp.
  - Scores row-tiled 2x on the PE: kTd [128, S] holds K duplicated on both
    partition halves; qd2 [128, S] holds a HEAD PAIR. Two concurrent K=64
    matmuls (tile_position (0,0)/(64,0)) fill one [128, 1024] PSUM pair ->
    a single [128, 1024] exp on ACT (the kernel bottleneck: ~220us/core of
    pure exp streaming).
  - AV via v_aug [128, 65] (ones column -> softmax denominator Z free in
    row 64). Z rows batched into one [128, 32] reciprocal via DMA
    gather/scatter; 1/Z broadcast with a K=1 PE matmul; DVE mul -> oT.
  - Cross-batch software pipelining by interleaved EMISSION: batch 1's
    projection work is emitted in small chunks inside batch 0's attention
    kc-loop (and batch 0's o-projection inside batch 1's attention), so
    the Tile scheduler's priority order alternates and the PE fills the
    ACT-bound gaps. PSUM tags: sc 4 banks, av 2, prj 1, pr 1.
  - DMA spread: enc on gpsimd, x on sync(+scalar at startup), weights on
    scalar, z-dance + stores on sync.
"""

import numpy as np
import ml_dtypes

import concourse.bass as bass
from concourse import bacc
import concourse.mybir as mybir
import concourse.tile as tile
from concourse.bass_utils import run_bass_kernel_spmd
from concourse.masks import make_identity

BF16 = ml_dtypes.bfloat16
F32 = mybir.dt.float32
BF = mybir.dt.bfloat16

B = 2
S = 2048
HID = 2048
D = 64          # head dim
CH = 4 * D      # 256 q channels per core
NCORES = 8
NH = HID // 128  # 16 hidden chunks
NKC = S // 128   # 16 key chunks of 128
NQC = S // 512   # 4 q tiles of 512
NST = S // 512   # 4 s tiles of 512
SCALE = 1.0 / np.sqrt(D)


def _build_nc() -> bass.Bass:
    nc = bacc.Bacc()

    xT = nc.dram_tensor("xT", [B, HID, S], BF, kind="ExternalInput")
    encT = nc.dram_tensor("encT", [B, HID, S], BF, kind="ExternalInput")
    wq = nc.dram_tensor("wq", [HID, CH], BF, kind="ExternalInput")
    wkv = nc.dram_tensor("wkv", [HID, 128], BF, kind="ExternalInput")
    wo = nc.dram_tensor("wo", [CH, HID], BF, kind="ExternalInput")
    bq = nc.dram_tensor("bq", [CH, 1], F32, kind="ExternalInput")
    bkv = nc.dram_tensor("bkv", [128, 1], F32, kind="ExternalInput")
    out = nc.dram_tensor("out", [B, S, HID], BF, kind="ExternalOutput")

    EXP = mybir.ActivationFunctionType.Exp

    with tile.TileContext(nc) as tc:
        with (
            tc.tile_pool(name="wpool", bufs=1) as wpool,
            tc.tile_pool(name="io", bufs=22) as io_pool,
            tc.tile_pool(name="acts", bufs=2) as acts,
            tc.tile_pool(name="vaug", bufs=2 * NKC) as vaug_pool,
            tc.tile_pool(name="epool", bufs=3) as epool,
            tc.tile_pool(name="avsb", bufs=8) as avsb_pool,
            tc.tile_pool(name="zp", bufs=2) as zpool,
            tc.tile_pool(name="osb", bufs=2) as osb_pool,
            tc.tile_pool(name="ps_sc", bufs=2, space="PSUM") as ps_sc,
            tc.tile_pool(name="ps_av", bufs=2, space="PSUM") as ps_av,
            tc.tile_pool(name="ps_prj", bufs=1, space="PSUM") as ps_prj,
            tc.tile_pool(name="ps_pr", bufs=1, space="PSUM") as ps_pr,
        ):
            # ---- resident weights / constants (scalar HWDGE queue) ----
            wkv_t = []
            for h in range(NH):
                wkvh = wpool.tile([128, 128], BF, name=f"wkv{h}")
                nc.scalar.dma_start(out=wkvh[:], in_=wkv[h * 128:(h + 1) * 128, :])
                wkv_t.append(wkvh)
            bkv_t = wpool.tile([128, 1], F32, name="bkv_t")
            nc.scalar.dma_start(out=bkv_t[:], in_=bkv[:, :])
            ident = wpool.tile([128, 128], BF, name="ident")
            make_identity(nc, ident[:])
            wq_t = []
            for h in range(NH):
                wqh = wpool.tile([128, CH], BF, name=f"wq{h}")
                nc.scalar.dma_start(out=wqh[:], in_=wq[h * 128:(h + 1) * 128, :])
                wq_t.append(wqh)
            bq_t = []
            for cc in range(2):
                bqc = wpool.tile([128, 1], F32, name=f"bq{cc}")
                nc.scalar.dma_start(out=bqc[:], in_=bq[cc * 128:(cc + 1) * 128, :])
                bq_t.append(bqc)
            wo_t = []
            for cc in range(2):
                woc = wpool.tile([128, HID], BF, name=f"wo{cc}")
                nc.scalar.dma_start(out=woc[:], in_=wo[cc * 128:(cc + 1) * 128, :])
                wo_t.append(woc)
            ones1 = wpool.tile([1, D], BF, name="ones1")
            nc.gpsimd.memset(ones1[:], 1.0)

            state = {}

            def proj_phase(b, startup):
                """Generator: KV proj, kTd/vT dup, v_aug, Q proj for batch b.
                Yields between small chunks so it can be pumped as PE filler
                inside the other batch's attention loop."""
                st_ = {}
                state[b] = st_
                # --- KV projection ---
                kvT = acts.tile([128, S], BF, tag="kvT", bufs=1, name=f"kvT{b}")
                for st in range(NST):
                    ssl = slice(st * 512, (st + 1) * 512)
                    ets = []
                    for h in range(NH):
                        et = io_pool.tile([128, 512], BF, tag="io",
                                          name=f"es{b}{st}{h}")
                        eng = nc.gpsimd if h % 2 == 0 else nc.sync
                        eng.dma_start(
                            out=et[:], in_=encT[b, h * 128:(h + 1) * 128, ssl])
                        ets.append(et)
                    yield
                    kvps = ps_prj.tile([128, 512], F32, tag="prj",
                                       name=f"kvp{b}{st}")
                    for h in range(NH):
                        nc.tensor.matmul(
                            kvps[:], wkv_t[h][:], ets[h][:],
                            start=(h == 0), stop=(h == NH - 1))
                        if h % 4 == 3:
                            yield
                    nc.vector.tensor_scalar_add(kvT[:, ssl], kvps[:], bkv_t[:])
                    yield
                # --- kTd (K duplicated on both halves), vT ---
                kTd = acts.tile([128, S], BF, tag="kTd", name=f"kTd{b}")
                vT = acts.tile([D, S], BF, tag="vT", bufs=1, name=f"vT{b}")
                nc.gpsimd.dma_start(out=kTd[0:D, :], in_=kvT[0:D, :])
                nc.gpsimd.dma_start(out=kTd[D:128, :], in_=kvT[0:D, :])
                nc.gpsimd.dma_start(out=vT[:], in_=kvT[D:128, :])
                st_["kTd"] = kTd
                yield
                # --- v_aug chunks [128, 65] with ones in col 64 ---
                v_aug = []
                for kc in range(NKC):
                    vtp = ps_pr.tile([128, D], BF, tag="pr", name=f"vtp{b}{kc}")
                    nc.tensor.transpose(
                        vtp[:], vT[:, kc * 128:(kc + 1) * 128], ident[0:D, 0:D])
                    va = vaug_pool.tile([128, D + 1], BF, tag=f"va{kc}",
                                        name=f"va{b}{kc}")
                    nc.gpsimd.memset(va[:, D:D + 1], 1.0)
                    nc.vector.tensor_copy(va[:, 0:D], vtp[:])
                    v_aug.append(va)
                    if kc % 4 == 3:
                        yield
                st_["va"] = v_aug
                # --- Q projection -> head-pair tiles qd2[hp] ---
                qd2 = [
                    acts.tile([128, S], BF, tag=f"qd{hp}", name=f"qd{b}{hp}")
                    for hp in range(2)
                ]
                st_["qd2"] = qd2
                for st in range(NST):
                    ssl = slice(st * 512, (st + 1) * 512)
                    xts = []
                    for h in range(NH):
                        xt = io_pool.tile([128, 512], BF, tag="io",
                                          name=f"xs{b}{st}{h}")
                        eng = nc.sync if (not startup or h % 2 == 0) else nc.scalar
                        eng.dma_start(
                            out=xt[:], in_=xT[b, h * 128:(h + 1) * 128, ssl])
                        xts.append(xt)
                    yield
                    for hp in range(2):
                        qps = ps_prj.tile([128, 512], F32, tag="prj",
                                          name=f"qp{b}{st}{hp}")
                        for h in range(NH):
                            nc.tensor.matmul(
                                qps[:], wq_t[h][:, hp * 128:(hp + 1) * 128],
                                xts[h][:],
                                start=(h == 0), stop=(h == NH - 1))
                            if h % 4 == 3:
                                yield
                        nc.vector.tensor_scalar_add(
                            qd2[hp][:, ssl], qps[:], bq_t[hp][:])
                        yield

            def pump(gen, n=1):
                if gen is None:
                    return None
                for _ in range(n):
                    try:
                        next(gen)
                    except StopIteration:
                        return None
                return gen

            def attn_phase(b, filler, pump_n=1):
                """Attention for batch b; pumps `filler` once per kc step."""
                st_ = state[b]
                kTd, v_aug, qd2 = st_["kTd"], st_["va"], st_["qd2"]
                oT_t = [
                    acts.tile([128, S], BF, tag=f"oT{hp}", name=f"oT{b}{hp}")
                    for hp in range(2)
                ]
                st_["oT"] = oT_t
                def norm_gen(av_sb, rrow, oT):
                    # deferred 1/Z broadcast + multiply: pumped inside the
                    # NEXT section's kc loop so the recip latency chain never
                    # head-of-line blocks the PE FIFO.
                    for qc, a0, a1 in av_sb:
                        qsl = slice(qc * 512, (qc + 1) * 512)
                        for j, av in ((0, a0), (1, a1)):
                            bc = ps_pr.tile([D, 512], F32, tag="pr",
                                            name=f"bc{b}{id(a0)}{j}")
                            nc.tensor.matmul(
                                bc[:], ones1[:], rrow[j][:, qsl],
                                start=True, stop=True)
                            nc.vector.tensor_mul(
                                oT[j * D:(j + 1) * D, qsl], av[0:D, :], bc[:])
                        yield

                pending = None
                for hp in range(2):
                    av_sb = []
                    zP = zpool.tile([128, 32], BF, tag="zP", name=f"zP{b}{hp}")
                    for qc in range(NQC):
                        qsl = slice(qc * 512, (qc + 1) * 512)
                        av0 = ps_av.tile([D + 1, 512], F32, tag="av",
                                         name=f"av0_{b}{hp}{qc}")
                        av1 = ps_av.tile([D + 1, 512], F32, tag="av",
                                         name=f"av1_{b}{hp}{qc}")
                        # scores+exp run one kc ahead of AV so the PE FIFO
                        # never head-of-line blocks on exp (PE reorders only
                        # LDWEIGHTS, not MATMULs).
                        e_ahead = {}

                        def emit_scores(kc):
                            ksl = slice(kc * 128, (kc + 1) * 128)
                            sc2 = ps_sc.tile([128, 1024], F32, tag="sc",
                                             name=f"sc{b}{hp}{qc}{kc}")
                            nc.tensor.matmul(
                                sc2[:, 0:512], kTd[0:D, ksl],
                                qd2[hp][0:D, qsl],
                                start=True, stop=True, tile_position=(0, 0))
                            nc.tensor.matmul(
                                sc2[:, 512:1024], kTd[D:128, ksl],
                                qd2[hp][D:128, qsl],
                                start=True, stop=True, tile_position=(64, 0))
                            e2 = epool.tile([128, 1024], BF, tag="e",
                                            name=f"e{b}{hp}{qc}{kc}")
                            nc.scalar.activation(
                                e2[:], sc2[:], EXP, scale=float(SCALE))
                            e_ahead[kc] = e2

                        emit_scores(0)
                        for kc in range(NKC):
                            if kc + 1 < NKC:
                                emit_scores(kc + 1)
                            e2 = e_ahead.pop(kc)
                            nc.tensor.matmul(
                                av0[:], v_aug[kc][:], e2[:, 0:512],
                                start=(kc == 0), stop=(kc == NKC - 1))
                            nc.tensor.matmul(
                                av1[:], v_aug[kc][:], e2[:, 512:1024],
                                start=(kc == 0), stop=(kc == NKC - 1))
                            if pending is not None:
                                pending = pump(pending)
                            else:
                                filler = pump(filler, pump_n)
                        a0 = avsb_pool.tile([D + 1, 512], BF, tag="avsb",
                                            name=f"a0_{b}{hp}{qc}")
                        a1 = avsb_pool.tile([D + 1, 512], BF, tag="avsb",
                                            name=f"a1_{b}{hp}{qc}")
                        nc.vector.tensor_copy(a0[:], av0[:])
                        nc.vector.tensor_copy(a1[:], av1[:])
                        nc.sync.dma_start(
                            out=zP[:, qc * 4:(qc + 1) * 4], in_=a0[D:D + 1, :])
                        nc.sync.dma_start(
                            out=zP[:, 16 + qc * 4:16 + (qc + 1) * 4],
                            in_=a1[D:D + 1, :])
                        av_sb.append((qc, a0, a1))

                    # one batched reciprocal -> [1, 2048] rows
                    rP = zpool.tile([128, 32], BF, tag="rP", name=f"rP{b}{hp}")
                    with nc.allow_low_precision("bf16 1/Z broadcast"):
                        nc.vector.reciprocal(rP[:], zP[:])
                    rrow = [
                        zpool.tile([1, S], BF, tag="rrow", name=f"rr{b}{hp}{j}")
                        for j in range(2)
                    ]
                    for j in range(2):
                        for qc in range(NQC):
                            nc.sync.dma_start(
                                out=rrow[j][:, qc * 512:(qc + 1) * 512],
                                in_=rP[:, j * 16 + qc * 4:j * 16 + (qc + 1) * 4])
                    pending = norm_gen(av_sb, rrow, oT_t[hp])
                # drain leftover filler; hand the last head-pair's deferred
                # normalization back to the caller to overlap with what's next
                while filler is not None:
                    filler = pump(filler)
                return pending

            def oproj_phase(b, final=False):
                """Generator: o-projection for batch b. In `final` mode (the
                un-overlapped tail) it borrows the wide sc PSUM slots and
                alternates casts between DVE and the now-idle ACT; in filler
                mode it drips through the pr/prj single-bank slots."""
                oT_t = state[b]["oT"]
                nprj = 0
                for sc16 in range(S // 128):
                    s128 = slice(sc16 * 128, (sc16 + 1) * 128)
                    ob = osb_pool.tile([128, HID], BF, tag="osb",
                                       name=f"ob{b}{sc16}")
                    if final:
                        for h2 in range(2):
                            ops = ps_sc.tile([128, 1024], F32, tag="sc",
                                             name=f"op{b}{sc16}{h2}")
                            for j in range(2):
                                hsl = slice((h2 * 2 + j) * 512,
                                            (h2 * 2 + j + 1) * 512)
                                osl = slice(j * 512, (j + 1) * 512)
                                nc.tensor.matmul(
                                    ops[:, osl], oT_t[0][:, s128],
                                    wo_t[0][:, hsl], start=True, stop=False)
                                nc.tensor.matmul(
                                    ops[:, osl], oT_t[1][:, s128],
                                    wo_t[1][:, hsl], start=False, stop=True)
                            dsl = slice(h2 * 1024, (h2 + 1) * 1024)
                            if h2 == 0:
                                nc.vector.tensor_copy(ob[:, dsl], ops[:])
                            else:
                                nc.scalar.copy(ob[:, dsl], ops[:])
                            yield
                    else:
                        for hc in range(HID // 512):
                            hsl = slice(hc * 512, (hc + 1) * 512)
                            tag = "pr" if nprj % 2 == 0 else "prj"
                            pool = ps_pr if nprj % 2 == 0 else ps_prj
                            nprj += 1
                            ops = pool.tile([128, 512], F32, tag=tag,
                                            name=f"op{b}{sc16}{hc}")
                            nc.tensor.matmul(
                                ops[:], oT_t[0][:, s128], wo_t[0][:, hsl],
                                start=True, stop=False)
                            nc.tensor.matmul(
                                ops[:], oT_t[1][:, s128], wo_t[1][:, hsl],
                                start=False, stop=True)
                            nc.vector.tensor_copy(ob[:, hsl], ops[:])
                            yield
                    nc.sync.dma_start(out=out[b, s128, :], in_=ob[:])
                    yield

            def chain(*gens):
                for g in gens:
                    if g is not None:
                        yield from g

            # ---- pipeline ----
            p0 = proj_phase(0, startup=True)
            while pump(p0) is not None:
                pass
            pend0 = attn_phase(0, filler=proj_phase(1, startup=False),
                               pump_n=1)
            o0 = chain(pend0, oproj_phase(0))
            pend1 = attn_phase(1, filler=o0, pump_n=1)
            o1 = chain(pend1, oproj_phase(1, final=True))
            while pump(o1) is not None:
                pass

    if not nc.is_finalized():
        nc.finalize()
    return nc


_NC = None
_RUN_KWARGS = {}
_LAST_RESULT = None


def _get_nc():
    global _NC
    if _NC is None:
        _NC = _build_nc()
    return _NC


def kernel(x, encoder_output, Wq, bq, Wk, bk, Wv, bv, Wo, bo):
    nc = _get_nc()
    xT = np.ascontiguousarray(
        np.asarray(x, np.float32).transpose(0, 2, 1)).astype(BF16)
    encT = np.ascontiguousarray(
        np.asarray(encoder_output, np.float32).transpose(0, 2, 1)).astype(BF16)
    Wq = np.asarray(Wq, np.float32)
    Wk = np.asarray(Wk, np.float32)
    Wv = np.asarray(Wv, np.float32)
    Wo = np.asarray(Wo, np.float32)
    bq = np.asarray(bq, np.float32)
    bk = np.asarray(bk, np.float32)
    bv = np.asarray(bv, np.float32)
    in_maps = []
    for c in range(NCORES):
        csl = slice(c * CH, (c + 1) * CH)
        gsl = slice(c * D, (c + 1) * D)
        in_maps.append({
            "xT": xT,
            "encT": encT,
            "wq": np.ascontiguousarray(Wq[:, csl]).astype(BF16),
            "wkv": np.ascontiguousarray(
                np.concatenate([Wk[:, gsl], Wv[:, gsl]], axis=1)).astype(BF16),
            "wo": np.ascontiguousarray(Wo[csl, :]).astype(BF16),
            "bq": np.ascontiguousarray(bq[csl].reshape(CH, 1)),
            "bkv": np.ascontiguousarray(
                np.concatenate([bk[gsl], bv[gsl]]).reshape(128, 1)),
        })
    res = run_bass_kernel_spmd(nc, in_maps, list(range(NCORES)), **_RUN_KWARGS)
    global _LAST_RESULT
    _LAST_RESULT = res
    total = np.zeros((B, S, HID), np.float32)
    for c in range(NCORES):
        total += res.results[c]["out"].astype(np.float32)
    return total + np.asarray(bo, np.float32)
